# revision 19
# baseline (speedup 1.0000x reference)
"""Trainium2 Bass kernel for the 'general' attention mechanism.

Reference computation (S=2048, B=32, H=1024):
    proj     = einsum('sbh,kh->sbk', encoder_outputs, W) + b    # [S,B,H]
    energies = einsum('bh,sbh->bs', decoder_hidden, proj)       # [B,S]
    out      = softmax(energies, axis=1)[:, None, :]            # [B,1,S]

Algebraic rewrite used here (exact up to fp reassociation):
    energies[b,s] = sum_h enc[s,b,h] * v[b,h] + dec[b].b
    with v = dec @ W. The dec[b].b term is constant over s and cancels in
    softmax, so it is dropped.  This turns a 137-GFLOP projection into a
    memory-bound stream of dot products (256 MB of encoder data).

Distribution: data-parallel over the batch dim, 4 batches per NeuronCore.
Each core receives (host-side prepared, all fp16):
    encT [4, 1024, 2048]  = enc[:, 4i:4i+4, :] transposed to [b, h, s]
    decT [1024, 4]        = dec[4i:4i+4, :].T
    W    [1024, 1024]     (replicated)
and computes v^T on-device via TensorE, then energies via TensorE matmuls
(contraction over h on the partition axis, s streaming as the moving free
dim), then softmax on-device, emitting out [4, 2048] fp32.

fp16 inputs halve the HBM stream (the bottleneck) and run the PE at
1 cycle/row instead of fp32's 4.  Measured end-to-end rel err ~3e-3 vs
the 2e-2 gate (softmax energies accumulate in fp32 PSUM).
"""

import numpy as np

B, S, H = 32, 2048, 1024
NCORES = 8
BPC = B // NCORES  # 4 batches per core
P = 128
HC = H // P  # 8 h-chunks
NMM = 512  # matmul moving free dim (one PSUM bank of fp32)
S2 = S // 2  # s elements per half (online-softmax pipeline granularity)
SCH = S2 // NMM  # 2 s-chunks per half
KP = 2  # h-chunks per DMA (one enc DMA covers KP h-chunks)

_COMPILED = {}
LAST_RESULT = None


def _install_ntff_shim():
    """Provide antenv.axon_hooks (missing in this image) so trace=True works.

    Replicates trn_agent_boot's ctypes NTFF hook against libaxon_pjrt.so.
    Harmless no-op if the module already exists or the .so is absent.
    """
    import sys

    try:
        import antenv.axon_hooks  # noqa: F401

        return
    except ImportError:
        pass
    import contextlib
    import ctypes
    import types

    so_path = "/opt/axon/libaxon_pjrt.so"
    mod = types.ModuleType("antenv.axon_hooks")
    _state = {"hook": None}

    def set_axon_ntff_profile_hook(h):
        _state["hook"] = h

    def get_axon_ntff_profile_hook():
        if _state["hook"] is not None:
            return _state["hook"]
        try:
            lib = ctypes.CDLL(so_path)
        except OSError:
            return None
        if not hasattr(lib, "axon_start_nrt_profile"):
            return None
        lib.axon_start_nrt_profile.argtypes = [
            ctypes.POINTER(ctypes.c_int64),
            ctypes.c_size_t,
        ]
        lib.axon_start_nrt_profile.restype = ctypes.c_int64
        lib.axon_stop_nrt_profile.argtypes = [ctypes.c_char_p]
        lib.axon_stop_nrt_profile.restype = ctypes.c_int64

        @contextlib.contextmanager
        def _hook(output_dir, device_ids):
            import jax

            jax.devices()
            if device_ids:
                ids = (ctypes.c_int64 * len(device_ids))(*device_ids)
                rc = lib.axon_start_nrt_profile(ids, len(device_ids))
            else:
                rc = lib.axon_start_nrt_profile(None, 0)
            if rc != 0:
                raise RuntimeError(f"axon_start_nrt_profile rc={rc}")
            try:
                yield
            finally:
                n = lib.axon_stop_nrt_profile(str(output_dir).encode())
                print(f"ntff profile: {n} file(s) written to {output_dir}")

        _state["hook"] = _hook
        return _hook

    mod.set_axon_ntff_profile_hook = set_axon_ntff_profile_hook
    mod.get_axon_ntff_profile_hook = get_axon_ntff_profile_hook
    sys.modules["antenv.axon_hooks"] = mod


def _build():
    import concourse.bass as bass
    import concourse.mybir as mybir
    import concourse.tile as tile
    from concourse import bacc

    f32 = mybir.dt.float32
    f16 = mybir.dt.float16

    nc = bacc.Bacc("TRN2", target_bir_lowering=False, debug=False)
    # Host pre-shapes everything so every DMA is a plain contiguous transfer:
    #   encH [2, BPC, 128, HC, S2] = enc shard as [half, b, p, h_chunk, s']  (fp16)
    #   decTr [128, HC, BPC] = dec shard^T grouped as [p, h_chunk, b]        (fp16)
    #   Wr   [128, HC, H]    = W grouped as [p, h_chunk, h']                 (fp16)
    encT = nc.dram_tensor("encT", [BPC, H, S], f16, kind="ExternalInput").ap()
    # W and dec^T ride in one DMA: line (p, hc) = [W[hc*128+p, :], decT[hc*128+p, :]]
    Wd = nc.dram_tensor("Wd", [P, HC, H + BPC], f16, kind="ExternalInput").ap()
    out = nc.dram_tensor("out", [BPC, S], f32, kind="ExternalOutput").ap()

    ExpF = mybir.ActivationFunctionType.Exp
    Max = mybir.AluOpType.max
    Min = mybir.AluOpType.min
    Mult = mybir.AluOpType.mult
    Add = mybir.AluOpType.add
    Sub = mybir.AluOpType.subtract

    with tile.TileContext(nc) as tc:
        with (
            tc.tile_pool(name="wpool", bufs=1) as wpool,
            tc.tile_pool(name="encp", bufs=24) as encp,
            tc.tile_pool(name="small", bufs=1) as small,
            tc.tile_pool(name="pvt", bufs=2, space="PSUM") as pvt,
            tc.tile_pool(name="pe", bufs=6, space="PSUM") as pep,
        ):
            # --- load W + dec^T in one full-rate DMA (2056-byte lines) ---
            w_sb = wpool.tile([P, HC, H + BPC], f16, name="w_sb")
            nc.sync.dma_start(w_sb[:], Wd[:])

            # --- v^T = (dec @ W)^T computed directly as [h', b] tiles ---
            # out[h', b] = sum_h W[h, h'] * dec[b, h]; lhsT = W chunk, rhs = decT chunk
            vt_sb = small.tile([P, HC, BPC], f16, name="vt_sb")
            for pc in range(HC):
                pv = pvt.tile([P, BPC], f32, name="pv")
                for hc in range(HC):
                    nc.tensor.matmul(
                        pv[:],
                        lhsT=w_sb[:, hc, pc * P : (pc + 1) * P],
                        rhs=w_sb[:, hc, H : H + BPC],
                        start=(hc == 0),
                        stop=(hc == HC - 1),
                    )
                nc.vector.tensor_copy(vt_sb[:, pc, :], pv[:])

            # --- stream encoder tiles; energies via [1, NMM] psum rows ---
            # Batch b's energies row lives at partition 32*b so every
            # psum->sbuf copy starts on a legal 32-aligned base.  Within each
            # b, hc is the OUTER matmul loop (SC psum banks accumulate across
            # hc) so b's copies fire right after b's last enc tile lands.
            SC = S // NMM
            energies = small.tile([P, S], f32, name="energies")
            for b in range(BPC):
                et = []
                for hc in range(HC):
                    t = encp.tile([P, S], f16, name="et")
                    nc.sync.dma_start(t[:], encT[b, hc * P : (hc + 1) * P, :])
                    et.append(t)
                pe = [pep.tile([1, NMM], f32, name="pe") for _ in range(SC)]
                for hc in range(HC):
                    for sc in range(SC):
                        nc.tensor.matmul(
                            pe[sc][:],
                            lhsT=vt_sb[:, hc, b : b + 1],
                            rhs=et[hc][:, sc * NMM : (sc + 1) * NMM],
                            start=(hc == 0),
                            stop=(hc == HC - 1),
                        )
                for sc in range(SC):
                    nc.any.tensor_copy(
                        energies[32 * b : 32 * b + 1, sc * NMM : (sc + 1) * NMM],
                        pe[sc][:],
                    )

            # --- softmax over s, split across engines to shorten the tail ---
            nm = small.tile([P, 1], f32, name="nm")
            nc.vector.tensor_reduce(
                nm[:], energies[:], axis=mybir.AxisListType.X, op=Max,
                negate=True,
            )
            expv = small.tile([P, S], f32, name="expv")
            zsum = small.tile([P, 1], f32, name="zsum")
            nc.scalar.activation(
                expv[:], energies[:], ExpF, bias=nm[:], scale=1.0,
                accum_out=zsum[:],
            )
            rs = small.tile([P, 1], f32, name="rs")
            nc.vector.reciprocal(rs[:], zsum[:])
            # normalize split: DVE | ACT work disjoint s-ranges (GpSimd is
            # ~15x slower at tensor_scalar -- keep it out of the tail)
            out_sb = small.tile([P, S], f32, name="out_sb")
            c1 = 1152
            nc.vector.tensor_scalar_mul(out_sb[:, :c1], expv[:, :c1], rs[:])
            nc.scalar.activation(
                out_sb[:, c1:], expv[:, c1:],
                mybir.ActivationFunctionType.Copy, bias=0.0, scale=rs[:],
            )

            # single DMA: rows 0/32/64/96 of out_sb via a partition-step AP
            nc.sync.dma_start(out[:], out_sb[0:P:32, :])

    nc.compile()
    return nc


def _get_nc():
    if "nc" not in _COMPILED:
        _COMPILED["nc"] = _build()
    return _COMPILED["nc"]


def kernel(decoder_hidden, encoder_outputs, W, b=None, **_ignored):
    global LAST_RESULT
    import time as _time

    _install_ntff_shim()
    from concourse.bass_utils import run_bass_kernel_spmd

    dec = np.asarray(decoder_hidden, dtype=np.float32).astype(np.float16)
    enc = np.asarray(encoder_outputs, dtype=np.float32).astype(np.float16)
    Wm = np.ascontiguousarray(np.asarray(W, dtype=np.float32).astype(np.float16))

    t0 = _time.time()
    nc = _get_nc()
    t1 = _time.time()
    # Wr[p, hc, n] = W[hc*128+p, n]
    Wr = Wm.reshape(HC, P, H).transpose(1, 0, 2)  # [P, HC, H]
    in_maps = []
    for i in range(NCORES):
        sl = slice(i * BPC, (i + 1) * BPC)
        encT_i = np.ascontiguousarray(enc[:, sl, :].transpose(1, 2, 0))  # [BPC,H,S]
        decT_i = dec[sl, :].T.reshape(HC, P, BPC).transpose(1, 0, 2)  # [P, HC, BPC]
        Wd_i = np.ascontiguousarray(
            np.concatenate([Wr, decT_i], axis=2)
        )  # [P, HC, H + BPC]
        in_maps.append({"encT": encT_i, "Wd": Wd_i})
    t2 = _time.time()
    print(f"[kernel] build+compile {t1 - t0:.1f}s, shard prep {t2 - t1:.1f}s", flush=True)

    import os as _os

    mode = _os.environ.get("BASS_DISPATCH", "spmd")
    if mode == "percore":
        import jax
        from concourse import bass2jax

        devices = jax.devices()[:NCORES]
        results = []
        for i in range(NCORES):
            with jax.default_device(devices[i]):
                r = bass2jax.run_bass_via_pjrt(nc, [in_maps[i]], n_cores=1)
            results.append(r[0])
        from concourse.bass_utils import BassKernelResults

        res = BassKernelResults(
            results=results,
            instructions_and_trace=None,
            profile_json=None,
            exec_time_ns=None,
        )
    else:
        res = run_bass_kernel_spmd(nc, in_maps, core_ids=list(range(NCORES)))
    print(f"[kernel] {mode} run {_time.time() - t2:.1f}s", flush=True)
    LAST_RESULT = res
    outs = [np.asarray(res.results[i]["out"]) for i in range(NCORES)]
    att = np.concatenate(outs, axis=0).reshape(B, 1, S).astype(np.float32)
    return att



# revision 22
# speedup vs baseline: 1.0929x; 1.0929x over previous
"""Trainium2 Bass kernel for the 'general' attention mechanism.

Reference computation (S=2048, B=32, H=1024):
    proj     = einsum('sbh,kh->sbk', encoder_outputs, W) + b    # [S,B,H]
    energies = einsum('bh,sbh->bs', decoder_hidden, proj)       # [B,S]
    out      = softmax(energies, axis=1)[:, None, :]            # [B,1,S]

Algebraic rewrite used here (exact up to fp reassociation):
    energies[b,s] = sum_h enc[s,b,h] * v[b,h] + dec[b].b
    with v = dec @ W. The dec[b].b term is constant over s and cancels in
    softmax, so it is dropped.  This turns a 137-GFLOP projection into a
    memory-bound stream of dot products (256 MB of encoder data).

Distribution: data-parallel over the batch dim, 4 batches per NeuronCore.
Each core receives (host-side prepared, all fp16):
    encT [4, 1024, 2048]  = enc[:, 4i:4i+4, :] transposed to [b, h, s]
    decT [1024, 4]        = dec[4i:4i+4, :].T
    W    [1024, 1024]     (replicated)
and computes v^T on-device via TensorE, then energies via TensorE matmuls
(contraction over h on the partition axis, s streaming as the moving free
dim), then softmax on-device, emitting out [4, 2048] fp32.

fp16 inputs halve the HBM stream (the bottleneck) and run the PE at
1 cycle/row instead of fp32's 4.  Measured end-to-end rel err ~3e-3 vs
the 2e-2 gate (softmax energies accumulate in fp32 PSUM).
"""

import numpy as np

B, S, H = 32, 2048, 1024
NCORES = 8
BPC = B // NCORES  # 4 batches per core
P = 128
HC = H // P  # 8 h-chunks
NMM = 512  # matmul moving free dim (one PSUM bank of fp32)
S2 = S // 2  # s elements per half (online-softmax pipeline granularity)
SCH = S2 // NMM  # 2 s-chunks per half
KP = 2  # h-chunks per DMA (one enc DMA covers KP h-chunks)

_COMPILED = {}
LAST_RESULT = None


def _install_ntff_shim():
    """Provide antenv.axon_hooks (missing in this image) so trace=True works.

    Replicates trn_agent_boot's ctypes NTFF hook against libaxon_pjrt.so.
    Harmless no-op if the module already exists or the .so is absent.
    """
    import sys

    try:
        import antenv.axon_hooks  # noqa: F401

        return
    except ImportError:
        pass
    import contextlib
    import ctypes
    import types

    so_path = "/opt/axon/libaxon_pjrt.so"
    mod = types.ModuleType("antenv.axon_hooks")
    _state = {"hook": None}

    def set_axon_ntff_profile_hook(h):
        _state["hook"] = h

    def get_axon_ntff_profile_hook():
        if _state["hook"] is not None:
            return _state["hook"]
        try:
            lib = ctypes.CDLL(so_path)
        except OSError:
            return None
        if not hasattr(lib, "axon_start_nrt_profile"):
            return None
        lib.axon_start_nrt_profile.argtypes = [
            ctypes.POINTER(ctypes.c_int64),
            ctypes.c_size_t,
        ]
        lib.axon_start_nrt_profile.restype = ctypes.c_int64
        lib.axon_stop_nrt_profile.argtypes = [ctypes.c_char_p]
        lib.axon_stop_nrt_profile.restype = ctypes.c_int64

        @contextlib.contextmanager
        def _hook(output_dir, device_ids):
            import jax

            jax.devices()
            if device_ids:
                ids = (ctypes.c_int64 * len(device_ids))(*device_ids)
                rc = lib.axon_start_nrt_profile(ids, len(device_ids))
            else:
                rc = lib.axon_start_nrt_profile(None, 0)
            if rc != 0:
                raise RuntimeError(f"axon_start_nrt_profile rc={rc}")
            try:
                yield
            finally:
                n = lib.axon_stop_nrt_profile(str(output_dir).encode())
                print(f"ntff profile: {n} file(s) written to {output_dir}")

        _state["hook"] = _hook
        return _hook

    mod.set_axon_ntff_profile_hook = set_axon_ntff_profile_hook
    mod.get_axon_ntff_profile_hook = get_axon_ntff_profile_hook
    sys.modules["antenv.axon_hooks"] = mod


def _build():
    import concourse.bass as bass
    import concourse.mybir as mybir
    import concourse.tile as tile
    from concourse import bacc

    f32 = mybir.dt.float32
    f16 = mybir.dt.float16

    nc = bacc.Bacc("TRN2", target_bir_lowering=False, debug=False)
    # Host pre-shapes everything so every DMA is a plain contiguous transfer:
    #   encH [2, BPC, 128, HC, S2] = enc shard as [half, b, p, h_chunk, s']  (fp16)
    #   decTr [128, HC, BPC] = dec shard^T grouped as [p, h_chunk, b]        (fp16)
    #   Wr   [128, HC, H]    = W grouped as [p, h_chunk, h']                 (fp16)
    # encC[half, b, k, p, hc2, s'] = enc[half*S2+s', b, (2k+hc2)*128+p]:
    # each (half, b, k) DMA chunk is a fully contiguous 512 KB dram block.
    KC = HC // KP
    encC = nc.dram_tensor(
        "encC", [2, BPC, KC, P, KP, S2], f16, kind="ExternalInput"
    ).ap()
    # W and dec^T ride in one DMA: line (p, hc) = [W[hc*128+p, :], decT[hc*128+p, :]]
    Wd = nc.dram_tensor("Wd", [P, HC, H + BPC], f16, kind="ExternalInput").ap()
    out = nc.dram_tensor("out", [BPC, S], f32, kind="ExternalOutput").ap()

    ExpF = mybir.ActivationFunctionType.Exp
    Max = mybir.AluOpType.max
    Min = mybir.AluOpType.min
    Mult = mybir.AluOpType.mult
    Add = mybir.AluOpType.add
    Sub = mybir.AluOpType.subtract

    with tile.TileContext(nc) as tc:
        with (
            tc.tile_pool(name="wpool", bufs=1) as wpool,
            tc.tile_pool(name="encp", bufs=24) as encp,
            tc.tile_pool(name="small", bufs=1) as small,
            tc.tile_pool(name="pvt", bufs=2, space="PSUM") as pvt,
            tc.tile_pool(name="pe", bufs=6, space="PSUM") as pep,
        ):
            # --- load W + dec^T in one full-rate DMA (2056-byte lines) ---
            w_sb = wpool.tile([P, HC, H + BPC], f16, name="w_sb")
            nc.sync.dma_start(w_sb[:], Wd[:])

            # --- v^T = (dec @ W)^T computed directly as [h', b] tiles ---
            # out[h', b] = sum_h W[h, h'] * dec[b, h]; lhsT = W chunk, rhs = decT chunk
            vt_sb = small.tile([P, HC, BPC], f16, name="vt_sb")
            for pc in range(HC):
                pv = pvt.tile([P, BPC], f32, name="pv")
                for hc in range(HC):
                    nc.tensor.matmul(
                        pv[:],
                        lhsT=w_sb[:, hc, pc * P : (pc + 1) * P],
                        rhs=w_sb[:, hc, H : H + BPC],
                        start=(hc == 0),
                        stop=(hc == HC - 1),
                    )
                nc.vector.tensor_copy(vt_sb[:, pc, :], pv[:])

            # --- stream encoder tiles; energies via [1, NMM] psum rows ---
            # Batch b's energies row lives at partition 32*b so every
            # psum->sbuf copy starts on a legal 32-aligned base.  Within each
            # b, hc is the OUTER matmul loop (SCH psum banks accumulate across
            # hc) so b's copies fire right after b's last enc tile lands.
            # Softmax is online over the two s-halves: half 0's max/exp/sum
            # run while half 1 streams, and the max fixup folds into the
            # final per-row normalization scalars.
            energ = [small.tile([P, S2], f32, name=f"energ{h}") for h in range(2)]
            expv = small.tile([P, S], f32, name="expv")
            out_sb = small.tile([P, S], f32, name="out_sb")
            nmax = [small.tile([P, 1], f32, name=f"nmax{h}") for h in range(2)]
            zsum = [small.tile([P, 1], f32, name=f"zsum{h}") for h in range(2)]
            for half in range(2):
                for b in range(BPC):
                    et = []
                    for k in range(KC):
                        t = encp.tile([P, KP, S2], f16, name="et")
                        nc.sync.dma_start(t[:], encC[half, b, k])
                        et.append(t)
                    pe = [pep.tile([1, NMM], f32, name="pe") for _ in range(SCH)]
                    for hc in range(HC):
                        for sc in range(SCH):
                            nc.tensor.matmul(
                                pe[sc][:],
                                lhsT=vt_sb[:, hc, b : b + 1],
                                rhs=et[hc // KP][:, hc % KP, sc * NMM : (sc + 1) * NMM],
                                start=(hc == 0),
                                stop=(hc == HC - 1),
                            )
                    for sc in range(SCH):
                        nc.any.tensor_copy(
                            energ[half][32 * b : 32 * b + 1, sc * NMM : (sc + 1) * NMM],
                            pe[sc][:],
                        )
                if half == 0:
                    nc.vector.tensor_reduce(
                        nmax[0][:], energ[0][:], axis=mybir.AxisListType.X, op=Max,
                        negate=True,
                    )
                    nc.scalar.activation(
                        expv[:, :S2], energ[0][:], ExpF, bias=nmax[0][:], scale=1.0,
                        accum_out=zsum[0][:],
                    )
                else:
                    nc.vector.tensor_reduce(
                        nmax[1][:], energ[1][:], axis=mybir.AxisListType.X, op=Max,
                        negate=True,
                    )
                    # global -max; fixup factor f0 = exp(nm - nm0) for half 0
                    nm = small.tile([P, 1], f32, name="nm")
                    nc.vector.tensor_scalar(nm[:], nmax[0][:], nmax[1][:], None, Min)
                    d0 = small.tile([P, 1], f32, name="d0")
                    nc.vector.tensor_scalar(d0[:], nm[:], nmax[0][:], None, Sub)
                    f0 = small.tile([P, 1], f32, name="f0")
                    nc.scalar.activation(f0[:], d0[:], ExpF, bias=0.0, scale=1.0)
                    nc.scalar.activation(
                        expv[:, S2:], energ[1][:], ExpF, bias=nm[:], scale=1.0,
                        accum_out=zsum[1][:],
                    )
                    # z = z0*f0 + z1 ; rs1 = 1/z ; rs0 = f0*rs1
                    z = small.tile([P, 1], f32, name="z")
                    nc.vector.scalar_tensor_tensor(
                        z[:], zsum[0][:], f0[:], zsum[1][:], op0=Mult, op1=Add
                    )
                    rs1 = small.tile([P, 1], f32, name="rs1")
                    nc.vector.reciprocal(rs1[:], z[:])
                    rs0 = small.tile([P, 1], f32, name="rs0")
                    nc.vector.tensor_scalar(rs0[:], f0[:], rs1[:], None, Mult)
                    # normalize: DVE takes half 1, ACT takes half 0 (GpSimd is
                    # ~15x slower at tensor_scalar -- keep it out of the tail)
                    nc.vector.tensor_scalar_mul(out_sb[:, S2:], expv[:, S2:], rs1[:])
                    nc.scalar.activation(
                        out_sb[:, :S2], expv[:, :S2],
                        mybir.ActivationFunctionType.Copy, bias=0.0, scale=rs0[:],
                    )

            # single DMA: rows 0/32/64/96 of out_sb via a partition-step AP
            nc.sync.dma_start(out[:], out_sb[0:P:32, :])

    nc.compile()
    return nc


def _get_nc():
    if "nc" not in _COMPILED:
        _COMPILED["nc"] = _build()
    return _COMPILED["nc"]


def kernel(decoder_hidden, encoder_outputs, W, b=None, **_ignored):
    global LAST_RESULT
    import time as _time

    _install_ntff_shim()
    from concourse.bass_utils import run_bass_kernel_spmd

    dec = np.asarray(decoder_hidden, dtype=np.float32).astype(np.float16)
    enc = np.asarray(encoder_outputs, dtype=np.float32).astype(np.float16)
    Wm = np.ascontiguousarray(np.asarray(W, dtype=np.float32).astype(np.float16))

    t0 = _time.time()
    nc = _get_nc()
    t1 = _time.time()
    # Wr[p, hc, n] = W[hc*128+p, n]
    Wr = Wm.reshape(HC, P, H).transpose(1, 0, 2)  # [P, HC, H]
    KC = HC // KP
    in_maps = []
    for i in range(NCORES):
        sl = slice(i * BPC, (i + 1) * BPC)
        # encC[half, b, k, p, hc2, s'] = enc[half*S2+s', 4i+b, (2k+hc2)*128+p]
        encC_i = np.ascontiguousarray(
            enc[:, sl, :].reshape(2, S2, BPC, KC, KP, P).transpose(0, 2, 3, 5, 4, 1)
        )  # [2, BPC, KC, P, KP, S2]
        decT_i = dec[sl, :].T.reshape(HC, P, BPC).transpose(1, 0, 2)  # [P, HC, BPC]
        Wd_i = np.ascontiguousarray(
            np.concatenate([Wr, decT_i], axis=2)
        )  # [P, HC, H + BPC]
        in_maps.append({"encC": encC_i, "Wd": Wd_i})
    t2 = _time.time()
    print(f"[kernel] build+compile {t1 - t0:.1f}s, shard prep {t2 - t1:.1f}s", flush=True)

    import os as _os

    mode = _os.environ.get("BASS_DISPATCH", "spmd")
    if mode == "percore":
        import jax
        from concourse import bass2jax

        devices = jax.devices()[:NCORES]
        results = []
        for i in range(NCORES):
            with jax.default_device(devices[i]):
                r = bass2jax.run_bass_via_pjrt(nc, [in_maps[i]], n_cores=1)
            results.append(r[0])
        from concourse.bass_utils import BassKernelResults

        res = BassKernelResults(
            results=results,
            instructions_and_trace=None,
            profile_json=None,
            exec_time_ns=None,
        )
    else:
        res = run_bass_kernel_spmd(nc, in_maps, core_ids=list(range(NCORES)))
    print(f"[kernel] {mode} run {_time.time() - t2:.1f}s", flush=True)
    LAST_RESULT = res
    outs = [np.asarray(res.results[i]["out"]) for i in range(NCORES)]
    att = np.concatenate(outs, axis=0).reshape(B, 1, S).astype(np.float32)
    return att



# revision 24
# speedup vs baseline: 1.1080x; 1.0138x over previous
"""Trainium2 Bass kernel for the 'general' attention mechanism.

Reference computation (S=2048, B=32, H=1024):
    proj     = einsum('sbh,kh->sbk', encoder_outputs, W) + b    # [S,B,H]
    energies = einsum('bh,sbh->bs', decoder_hidden, proj)       # [B,S]
    out      = softmax(energies, axis=1)[:, None, :]            # [B,1,S]

Algebraic rewrite used here (exact up to fp reassociation):
    energies[b,s] = sum_h enc[s,b,h] * v[b,h] + dec[b].b
    with v = dec @ W. The dec[b].b term is constant over s and cancels in
    softmax, so it is dropped.  This turns a 137-GFLOP projection into a
    memory-bound stream of dot products (256 MB of encoder data).

Distribution: data-parallel over the batch dim, 4 batches per NeuronCore.
Each core receives (host-side prepared, all fp16):
    encC [2, 4, 4, 128, 2, 1024] = enc shard as [s_half, b, hc_pair, p, hc2, s']
                                   (every DMA chunk is a contiguous 512 KB block)
    Wd   [128, 8, 1028]          = W (replicated) with dec^T appended per line
It computes v^T = (dec @ W)^T on-device via TensorE, then energies via
TensorE matmuls (contraction over h on the partition axis, s streaming as
the moving free dim, fp32 PSUM accumulation), then an online softmax over
the two s-halves: half 0's max/exp/sum run while half 1 is still
streaming, and the global-max fixup folds into the final per-row
normalization scalars.  Output leaves in one partition-strided DMA.

fp16 inputs halve the HBM stream (the bottleneck: ~17 MB/core of encoder
data) and run the PE at 1 cycle/row instead of fp32's 4.  Measured
end-to-end rel err ~3.2e-3 vs the 2e-2 gate.
"""

import numpy as np

B, S, H = 32, 2048, 1024
NCORES = 8
BPC = B // NCORES  # 4 batches per core
P = 128
HC = H // P  # 8 h-chunks
NMM = 512  # matmul moving free dim (one PSUM bank of fp32)
S2 = S // 2  # s elements per half (online-softmax pipeline granularity)
SCH = S2 // NMM  # 2 s-chunks per half
KP = 2  # h-chunks per DMA (one enc DMA covers KP h-chunks)

_COMPILED = {}
LAST_RESULT = None


def _install_ntff_shim():
    """Provide antenv.axon_hooks (missing in this image) so trace=True works.

    Replicates trn_agent_boot's ctypes NTFF hook against libaxon_pjrt.so.
    Harmless no-op if the module already exists or the .so is absent.
    """
    import sys

    try:
        import antenv.axon_hooks  # noqa: F401

        return
    except ImportError:
        pass
    import contextlib
    import ctypes
    import types

    so_path = "/opt/axon/libaxon_pjrt.so"
    mod = types.ModuleType("antenv.axon_hooks")
    _state = {"hook": None}

    def set_axon_ntff_profile_hook(h):
        _state["hook"] = h

    def get_axon_ntff_profile_hook():
        if _state["hook"] is not None:
            return _state["hook"]
        try:
            lib = ctypes.CDLL(so_path)
        except OSError:
            return None
        if not hasattr(lib, "axon_start_nrt_profile"):
            return None
        lib.axon_start_nrt_profile.argtypes = [
            ctypes.POINTER(ctypes.c_int64),
            ctypes.c_size_t,
        ]
        lib.axon_start_nrt_profile.restype = ctypes.c_int64
        lib.axon_stop_nrt_profile.argtypes = [ctypes.c_char_p]
        lib.axon_stop_nrt_profile.restype = ctypes.c_int64

        @contextlib.contextmanager
        def _hook(output_dir, device_ids):
            import jax

            jax.devices()
            if device_ids:
                ids = (ctypes.c_int64 * len(device_ids))(*device_ids)
                rc = lib.axon_start_nrt_profile(ids, len(device_ids))
            else:
                rc = lib.axon_start_nrt_profile(None, 0)
            if rc != 0:
                raise RuntimeError(f"axon_start_nrt_profile rc={rc}")
            try:
                yield
            finally:
                n = lib.axon_stop_nrt_profile(str(output_dir).encode())
                print(f"ntff profile: {n} file(s) written to {output_dir}")

        _state["hook"] = _hook
        return _hook

    mod.set_axon_ntff_profile_hook = set_axon_ntff_profile_hook
    mod.get_axon_ntff_profile_hook = get_axon_ntff_profile_hook
    sys.modules["antenv.axon_hooks"] = mod


def _build():
    import concourse.bass as bass
    import concourse.mybir as mybir
    import concourse.tile as tile
    from concourse import bacc

    f32 = mybir.dt.float32
    f16 = mybir.dt.float16

    nc = bacc.Bacc("TRN2", target_bir_lowering=False, debug=False)
    # Host pre-shapes everything so every DMA is a plain contiguous transfer:
    #   encH [2, BPC, 128, HC, S2] = enc shard as [half, b, p, h_chunk, s']  (fp16)
    #   decTr [128, HC, BPC] = dec shard^T grouped as [p, h_chunk, b]        (fp16)
    #   Wr   [128, HC, H]    = W grouped as [p, h_chunk, h']                 (fp16)
    # encC[half, b, k, p, hc2, s'] = enc[half*S2+s', b, (2k+hc2)*128+p]:
    # each (half, b, k) DMA chunk is a fully contiguous 512 KB dram block.
    KC = HC // KP
    encC = nc.dram_tensor(
        "encC", [2, BPC, KC, P, KP, S2], f16, kind="ExternalInput"
    ).ap()
    # W and dec^T ride in one DMA: line (p, hc) = [W[hc*128+p, :], decT[hc*128+p, :]]
    Wd = nc.dram_tensor("Wd", [P, HC, H + BPC], f16, kind="ExternalInput").ap()
    out = nc.dram_tensor("out", [BPC, S], f32, kind="ExternalOutput").ap()

    ExpF = mybir.ActivationFunctionType.Exp
    Max = mybir.AluOpType.max
    Min = mybir.AluOpType.min
    Mult = mybir.AluOpType.mult
    Add = mybir.AluOpType.add
    Sub = mybir.AluOpType.subtract

    with tile.TileContext(nc) as tc:
        with (
            tc.tile_pool(name="wpool", bufs=1) as wpool,
            tc.tile_pool(name="encp", bufs=24) as encp,
            tc.tile_pool(name="small", bufs=1) as small,
            tc.tile_pool(name="pvt", bufs=2, space="PSUM") as pvt,
            tc.tile_pool(name="pe", bufs=6, space="PSUM") as pep,
        ):
            # --- load W + dec^T in one full-rate DMA (2056-byte lines) ---
            w_sb = wpool.tile([P, HC, H + BPC], f16, name="w_sb")
            nc.sync.dma_start(w_sb[:], Wd[:])

            # --- v^T = (dec @ W)^T computed directly as [h', b] tiles ---
            # out[h', b] = sum_h W[h, h'] * dec[b, h]; lhsT = W chunk, rhs = decT chunk
            vt_sb = small.tile([P, HC, BPC], f16, name="vt_sb")
            for pc in range(HC):
                pv = pvt.tile([P, BPC], f32, name="pv")
                for hc in range(HC):
                    nc.tensor.matmul(
                        pv[:],
                        lhsT=w_sb[:, hc, pc * P : (pc + 1) * P],
                        rhs=w_sb[:, hc, H : H + BPC],
                        start=(hc == 0),
                        stop=(hc == HC - 1),
                    )
                nc.vector.tensor_copy(vt_sb[:, pc, :], pv[:])

            # --- stream encoder tiles; energies via [1, NMM] psum rows ---
            # Batch b's energies row lives at partition 32*b so every
            # psum->sbuf copy starts on a legal 32-aligned base.  Within each
            # b, hc is the OUTER matmul loop (SCH psum banks accumulate across
            # hc) so b's copies fire right after b's last enc tile lands.
            # Softmax is online over the two s-halves: half 0's max/exp/sum
            # run while half 1 streams, and the max fixup folds into the
            # final per-row normalization scalars.
            energ = [small.tile([P, S2], f32, name=f"energ{h}") for h in range(2)]
            expv = small.tile([P, S], f32, name="expv")
            out_sb = small.tile([P, S], f32, name="out_sb")
            nmax = [small.tile([P, 1], f32, name=f"nmax{h}") for h in range(2)]
            zsum = [small.tile([P, 1], f32, name=f"zsum{h}") for h in range(2)]
            for half in range(2):
                for b in range(BPC):
                    et = []
                    for k in range(KC):
                        t = encp.tile([P, KP, S2], f16, name="et")
                        nc.sync.dma_start(t[:], encC[half, b, k])
                        et.append(t)
                    pe = [pep.tile([1, NMM], f32, name="pe") for _ in range(SCH)]
                    for hc in range(HC):
                        for sc in range(SCH):
                            nc.tensor.matmul(
                                pe[sc][:],
                                lhsT=vt_sb[:, hc, b : b + 1],
                                rhs=et[hc // KP][:, hc % KP, sc * NMM : (sc + 1) * NMM],
                                start=(hc == 0),
                                stop=(hc == HC - 1),
                            )
                    for sc in range(SCH):
                        nc.any.tensor_copy(
                            energ[half][32 * b : 32 * b + 1, sc * NMM : (sc + 1) * NMM],
                            pe[sc][:],
                        )
                if half == 0:
                    nc.vector.tensor_reduce(
                        nmax[0][:], energ[0][:], axis=mybir.AxisListType.X, op=Max,
                        negate=True,
                    )
                    nc.scalar.activation(
                        expv[:, :S2], energ[0][:], ExpF, bias=nmax[0][:], scale=1.0,
                        accum_out=zsum[0][:],
                    )
                else:
                    nc.vector.tensor_reduce(
                        nmax[1][:], energ[1][:], axis=mybir.AxisListType.X, op=Max,
                        negate=True,
                    )
                    # global -max; fixup factor f0 = exp(nm - nm0) for half 0
                    # (d0 = nm - nm0 = min(nm1 - nm0, 0), fused off nm's path)
                    nm = small.tile([P, 1], f32, name="nm")
                    nc.vector.tensor_scalar(nm[:], nmax[0][:], nmax[1][:], None, Min)
                    d0 = small.tile([P, 1], f32, name="d0")
                    nc.vector.tensor_scalar(
                        d0[:], nmax[1][:], nmax[0][:], 0.0, Sub, Min
                    )
                    f0 = small.tile([P, 1], f32, name="f0")
                    nc.scalar.activation(f0[:], d0[:], ExpF, bias=0.0, scale=1.0)
                    nc.scalar.activation(
                        expv[:, S2:], energ[1][:], ExpF, bias=nm[:], scale=1.0,
                        accum_out=zsum[1][:],
                    )
                    # z = z0*f0 + z1 ; rs1 = 1/z ; rs0 = f0*rs1
                    z = small.tile([P, 1], f32, name="z")
                    nc.vector.scalar_tensor_tensor(
                        z[:], zsum[0][:], f0[:], zsum[1][:], op0=Mult, op1=Add
                    )
                    rs1 = small.tile([P, 1], f32, name="rs1")
                    nc.vector.reciprocal(rs1[:], z[:])
                    rs0 = small.tile([P, 1], f32, name="rs0")
                    nc.vector.tensor_scalar(rs0[:], f0[:], rs1[:], None, Mult)
                    # normalize: DVE takes half 1, ACT takes half 0 (GpSimd is
                    # ~15x slower at tensor_scalar -- keep it out of the tail)
                    nc.vector.tensor_scalar_mul(out_sb[:, S2:], expv[:, S2:], rs1[:])
                    nc.scalar.activation(
                        out_sb[:, :S2], expv[:, :S2],
                        mybir.ActivationFunctionType.Copy, bias=0.0, scale=rs0[:],
                    )

            # single DMA: rows 0/32/64/96 of out_sb via a partition-step AP
            nc.sync.dma_start(out[:], out_sb[0:P:32, :])

    nc.compile()
    return nc


def _get_nc():
    if "nc" not in _COMPILED:
        _COMPILED["nc"] = _build()
    return _COMPILED["nc"]


def kernel(decoder_hidden, encoder_outputs, W, b=None, **_ignored):
    global LAST_RESULT
    import time as _time

    _install_ntff_shim()
    from concourse.bass_utils import run_bass_kernel_spmd

    dec = np.asarray(decoder_hidden, dtype=np.float32).astype(np.float16)
    enc = np.asarray(encoder_outputs, dtype=np.float32).astype(np.float16)
    Wm = np.ascontiguousarray(np.asarray(W, dtype=np.float32).astype(np.float16))

    t0 = _time.time()
    nc = _get_nc()
    t1 = _time.time()
    # Wr[p, hc, n] = W[hc*128+p, n]
    Wr = Wm.reshape(HC, P, H).transpose(1, 0, 2)  # [P, HC, H]
    KC = HC // KP
    in_maps = []
    for i in range(NCORES):
        sl = slice(i * BPC, (i + 1) * BPC)
        # encC[half, b, k, p, hc2, s'] = enc[half*S2+s', 4i+b, (2k+hc2)*128+p]
        encC_i = np.ascontiguousarray(
            enc[:, sl, :].reshape(2, S2, BPC, KC, KP, P).transpose(0, 2, 3, 5, 4, 1)
        )  # [2, BPC, KC, P, KP, S2]
        decT_i = dec[sl, :].T.reshape(HC, P, BPC).transpose(1, 0, 2)  # [P, HC, BPC]
        Wd_i = np.ascontiguousarray(
            np.concatenate([Wr, decT_i], axis=2)
        )  # [P, HC, H + BPC]
        in_maps.append({"encC": encC_i, "Wd": Wd_i})
    t2 = _time.time()
    print(f"[kernel] build+compile {t1 - t0:.1f}s, shard prep {t2 - t1:.1f}s", flush=True)

    import os as _os

    mode = _os.environ.get("BASS_DISPATCH", "spmd")
    if mode == "percore":
        import jax
        from concourse import bass2jax

        devices = jax.devices()[:NCORES]
        results = []
        for i in range(NCORES):
            with jax.default_device(devices[i]):
                r = bass2jax.run_bass_via_pjrt(nc, [in_maps[i]], n_cores=1)
            results.append(r[0])
        from concourse.bass_utils import BassKernelResults

        res = BassKernelResults(
            results=results,
            instructions_and_trace=None,
            profile_json=None,
            exec_time_ns=None,
        )
    else:
        res = run_bass_kernel_spmd(nc, in_maps, core_ids=list(range(NCORES)))
    print(f"[kernel] {mode} run {_time.time() - t2:.1f}s", flush=True)
    LAST_RESULT = res
    outs = [np.asarray(res.results[i]["out"]) for i in range(NCORES)]
    att = np.concatenate(outs, axis=0).reshape(B, 1, S).astype(np.float32)
    return att

